# revision 16
# baseline (speedup 1.0000x reference)
"""Trainium2 Bass kernel for BasicBahdanauAttnDecoder.

Strategy (8 NeuronCores, no collectives):
  - Data-parallel over batch B=32 -> 4 batch elements per core.
  - Each core runs: embedding gather (indirect DMA), annot/P0 precompute,
    the sequential T=64 LSTM+attention scan, penultimate projection,
    vocab projection (tied emb weights) + log_softmax, all locally.
  - Host assembles full outputs by concatenating per-core batch slices.

Numerics: bf16 weights/activations for matmuls (PSUM accumulates fp32),
fp32 cell states and softmax statistics. Vocab logsumexp uses no max-shift
(logits are O(1), exp is safe in fp32).

Layouts ("T" suffix = transposed, contraction dim on partitions):
  - scan activations: [dim -> 128-partition tiles, batch(4) free]
  - stackT16 [128, 12, 64, 4]: per-step [h1n; ctx] bf16 (feeds next step + pen)
  - vocab: penT16 [128E, 4kt, 256tok] stationary, embT bf16 moving.
"""

import os
import numpy as np
import ml_dtypes

import concourse.bass as bass
import concourse.bacc as bacc
import concourse.mybir as mybir
import concourse.tile as tile
from concourse.bass_utils import run_bass_kernel_spmd
from concourse.masks import make_identity

BF16 = mybir.dt.bfloat16
F32 = mybir.dt.float32
I32 = mybir.dt.int32
AF = mybir.ActivationFunctionType
ALU = mybir.AluOpType

T, B, S = 64, 32, 64
V, E, H = 32000, 512, 512
NCORES = 8
BL = B // NCORES          # 4 batch per core
TOK = T * BL              # 256 tokens per core
G = 4 * H                 # 2048 gates
DC = 2 * H                # 1024 ctx dim
NMG = G // 128            # 16
NKC = DC // 128           # 8
NKH = H // 128            # 4
NME = E // 128            # 4
NKP = (3 * H) // 128      # 12
# vocab chunks: 62 x 512 + 1 x 256
VCH = [(i * 512, 512) for i in range(62)] + [(31744, 256)]

bf = ml_dtypes.bfloat16


def build_nc():
    # Bacc (not plain Bass): its finalize() splits multi-sem waits into
    # EventSemaphore instructions, which walrus codegen requires.
    nc = bacc.Bacc("TRN2", target_bir_lowering=False)

    # ---------------- I/O declarations ----------------
    d_idx = nc.declare_dram_parameter("idx", [128, 2], I32, isOutput=False)
    d_emb16 = nc.declare_dram_parameter("emb16", [V, E], BF16, isOutput=False)
    d_embT16 = nc.declare_dram_parameter("embT16", [E, V], BF16, isOutput=False)
    d_bout16 = nc.declare_dram_parameter("bout16", [1, V], BF16, isOutput=False)
    d_wctxT = nc.declare_dram_parameter("wctxT", [DC, G], BF16, isOutput=False)
    d_wembT = nc.declare_dram_parameter("wembT", [E, G], BF16, isOutput=False)
    d_whh0T = nc.declare_dram_parameter("whh0T", [H, G], BF16, isOutput=False)
    d_wih1T = nc.declare_dram_parameter("wih1T", [H, G], BF16, isOutput=False)
    d_whh1T = nc.declare_dram_parameter("whh1T", [H, G], BF16, isOutput=False)
    d_whT = nc.declare_dram_parameter("whT", [H, H], BF16, isOutput=False)
    d_waT = nc.declare_dram_parameter("waT", [DC, H], BF16, isOutput=False)
    d_woT = nc.declare_dram_parameter("woT", [H, 1], BF16, isOutput=False)
    d_wpT = nc.declare_dram_parameter("wpT", [3 * H, E], BF16, isOutput=False)
    d_bias0 = nc.declare_dram_parameter("bias0", [128, NMG], F32, isOutput=False)
    d_bias1 = nc.declare_dram_parameter("bias1", [128, NMG], F32, isOutput=False)
    d_biasa = nc.declare_dram_parameter("biasa", [128, NKH], F32, isOutput=False)
    d_bp = nc.declare_dram_parameter("bp", [128, NME], F32, isOutput=False)
    d_es16 = nc.declare_dram_parameter("es16", [S, BL, DC], BF16, isOutput=False)
    d_esT16 = nc.declare_dram_parameter("esT16", [DC, S * BL], BF16, isOutput=False)

    d_scores = nc.declare_dram_parameter("scores_o", [TOK, V], F32, isOutput=True)
    # stored [layer, ktile, partition, b]; host reorders to [2, BL, H]
    d_h = nc.declare_dram_parameter("h_o", [2, NKH, 128, BL], F32, isOutput=True)
    d_c = nc.declare_dram_parameter("c_o", [2, NKH, 128, BL], F32, isOutput=True)

    with tile.TileContext(nc) as tc:
        with tc.tile_pool(name="persist", bufs=1) as pp:
            ident16 = pp.tile([128, 128], BF16)
            make_identity(nc, ident16[:, :])
            penT16 = pp.tile([128, NME, TOK], BF16)
            # final states (bf16 h, fp32 c) saved here at t=T-1
            h0fin = pp.tile([128, NKH, BL], F32)
            h1fin = pp.tile([128, NKH, BL], F32)
            c0fin = pp.tile([128, NKH, BL], F32)
            c1fin = pp.tile([128, NKH, BL], F32)

            with tc.tile_pool(name="scanw", bufs=1) as wp:
                # -------- load weights / constants into SBUF --------
                def load_w(dram, nk, ncol, tag):
                    t_ = wp.tile([128, nk, ncol], BF16, tag=tag)
                    nc.sync.dma_start(
                        out=t_[:, :, :],
                        in_=dram[:, :].rearrange("(k p) g -> p k g", p=128),
                    )
                    return t_

                wctx_sb = load_w(d_wctxT, NKC, G, "wctx")
                wemb_sb = load_w(d_wembT, NME, G, "wemb")
                whh0_sb = load_w(d_whh0T, NKH, G, "whh0")
                wih1_sb = load_w(d_wih1T, NKH, G, "wih1")
                whh1_sb = load_w(d_whh1T, NKH, G, "whh1")
                wh_sb = load_w(d_whT, NKH, H, "wh")
                wa_sb = load_w(d_waT, NKC, H, "wa")
                wo_sb = load_w(d_woT, NKH, 1, "wo")
                wp_sb = load_w(d_wpT, NKP, E, "wp")

                # stage small pointer-operand tiles through DVE so downstream
                # TensorScalarPtr/Activation ops carry fewer sync waits
                def load_small(dram, ncol, tag):
                    raw = wp.tile([128, ncol], F32, tag=tag + "_r")
                    nc.sync.dma_start(out=raw[:, :], in_=dram[:, :])
                    st = wp.tile([128, ncol], F32, tag=tag)
                    nc.vector.tensor_copy(out=st[:, :], in_=raw[:, :])
                    return st

                bias0_sb = load_small(d_bias0, NMG, "bias0")
                bias1_sb = load_small(d_bias1, NMG, "bias1")
                biasa_sb = load_small(d_biasa, NKH, "biasa")
                bp_sb = load_small(d_bp, NME, "bp")

                idx_sb = wp.tile([128, 2], I32)
                nc.sync.dma_start(out=idx_sb[:, :], in_=d_idx[:, :])
                es_sb = wp.tile([S, BL, NKC, 128], BF16)
                nc.sync.dma_start(
                    out=es_sb[:, :, :, :],
                    in_=d_es16[:, :, :].rearrange("s b (k d) -> s b k d", d=128),
                )
                esT_sb = wp.tile([128, NKC, S * BL], BF16)
                nc.sync.dma_start(
                    out=esT_sb[:, :, :],
                    in_=d_esT16[:, :].rearrange("(k p) n -> p k n", p=128),
                )

                # -------- precompute: gather + transpose trg_emb --------
                trg16 = wp.tile([128, 2, E], BF16)
                for mt in range(2):
                    nc.gpsimd.indirect_dma_start(
                        out=trg16[:, mt, :],
                        out_offset=None,
                        in_=d_emb16[:, :],
                        in_offset=bass.IndirectOffsetOnAxis(
                            ap=idx_sb[:, mt : mt + 1], axis=0
                        ),
                    )
                trgT16 = wp.tile([128, NME, TOK], BF16)
                with tc.tile_pool(name="preps", bufs=2, space="PSUM") as prep:
                    for mt in range(2):
                        for ke in range(NME):
                            tp = prep.tile([128, 128], BF16, tag="tp")
                            nc.tensor.transpose(
                                out=tp[:, :],
                                in_=trg16[:, mt, ke * 128 : (ke + 1) * 128],
                                identity=ident16[:, :],
                            )
                            nc.vector.tensor_copy(
                                out=trgT16[:, ke, mt * 128 : (mt + 1) * 128],
                                in_=tp[:, :],
                            )

                    # -------- P0[t] = trg_emb @ W_emb.T + b0  (fp32) --------
                    p0 = wp.tile([128, NMG, T, BL], F32)
                    for m in range(NMG):
                        ps = prep.tile([128, TOK], F32, tag="p0ps")
                        for k in range(NME):
                            nc.tensor.matmul(
                                out=ps[:, :],
                                lhsT=wemb_sb[:, k, m * 128 : (m + 1) * 128],
                                rhs=trgT16[:, k, :],
                                start=(k == 0),
                                stop=(k == NME - 1),
                            )
                        nc.vector.tensor_scalar_add(
                            out=p0[:, m, :, :].rearrange("p t b -> p (t b)"),
                            in0=ps[:, :],
                            scalar1=bias0_sb[:, m : m + 1],
                        )

                    # -------- annT = Wa @ es.T + (ba + bh)  (fp32) --------
                    annT = wp.tile([128, NKH, S, BL], F32)
                    for m in range(NKH):
                        ps = prep.tile([128, S * BL], F32, tag="aps")
                        for k in range(NKC):
                            nc.tensor.matmul(
                                out=ps[:, :],
                                lhsT=wa_sb[:, k, m * 128 : (m + 1) * 128],
                                rhs=esT_sb[:, k, :],
                                start=(k == 0),
                                stop=(k == NKC - 1),
                            )
                        nc.vector.tensor_scalar_add(
                            out=annT[:, m, :, :].rearrange("p s b -> p (s b)"),
                            in0=ps[:, :],
                            scalar1=biasa_sb[:, m : m + 1],
                        )

                # -------- the scan --------
                stackT = wp.tile([128, NKP, T, BL], BF16)
                zeros16 = wp.tile([128, NKP, BL], BF16)
                nc.vector.memset(zeros16[:, :, :], 0.0)
                h0z = wp.tile([128, NKH, BL], BF16)
                nc.vector.memset(h0z[:, :, :], 0.0)
                czero = wp.tile([128, NKH, BL], F32)
                nc.vector.memset(czero[:, :, :], 0.0)

                h0prev = h0z
                c0prev, c1prev = czero, czero

                with tc.tile_pool(name="scansb", bufs=1) as sp, tc.tile_pool(
                    name="scanps", bufs=1, space="PSUM"
                ) as pps:
                    for t in range(T):
                        if t == 0:
                            x_prev = zeros16  # [128, 12, BL]: h1|ctx all zero
                        else:
                            x_prev = stackT[:, :, t - 1, :]

                        # ---- LSTM layer 0 gates ----
                        gps = pps.tile([128, NMG, BL], F32, tag="gps", bufs=2)
                        for m in range(NMG):
                            for k in range(NKC):
                                nc.tensor.matmul(
                                    out=gps[:, m, :],
                                    lhsT=wctx_sb[:, k, m * 128 : (m + 1) * 128],
                                    rhs=x_prev[:, NKH + k, :],
                                    start=(k == 0),
                                    stop=False,
                                )
                            for k in range(NKH):
                                nc.tensor.matmul(
                                    out=gps[:, m, :],
                                    lhsT=whh0_sb[:, k, m * 128 : (m + 1) * 128],
                                    rhs=h0prev[:, k, :],
                                    start=False,
                                    stop=(k == NKH - 1),
                                )
                        nc.vector.tensor_tensor(
                            out=gps[:, :, :],
                            in0=gps[:, :, :],
                            in1=p0[:, :, t, :],
                            op=ALU.add,
                        )

                        def lstm_elem(gtile, cprev, hname, cname, to_stack=None):
                            # gtile [128, 16, BL] psum (i,f,g,o); returns (h16, cnew)
                            sA = sp.tile([128, NKH, BL], F32, tag="sA", bufs=2)
                            sB = sp.tile([128, NKH, BL], F32, tag="sB", bufs=2)
                            sC = sp.tile([128, NKH, BL], F32, tag="sC", bufs=2)
                            sD = sp.tile([128, NKH, BL], F32, tag="sD", bufs=2)
                            nc.scalar.activation(sA[:, :, :], gtile[:, 0:4, :], AF.Sigmoid)
                            nc.scalar.activation(sB[:, :, :], gtile[:, 4:8, :], AF.Sigmoid)
                            nc.scalar.activation(sC[:, :, :], gtile[:, 8:12, :], AF.Tanh)
                            nc.scalar.activation(sD[:, :, :], gtile[:, 12:16, :], AF.Sigmoid)
                            nc.vector.tensor_tensor(
                                out=sA[:, :, :], in0=sA[:, :, :], in1=sC[:, :, :],
                                op=ALU.mult,
                            )
                            cnew = sp.tile([128, NKH, BL], F32, tag=cname, bufs=2)
                            nc.vector.tensor_tensor(
                                out=cnew[:, :, :], in0=sB[:, :, :], in1=cprev[:, :, :],
                                op=ALU.mult,
                            )
                            nc.vector.tensor_tensor(
                                out=cnew[:, :, :], in0=cnew[:, :, :], in1=sA[:, :, :],
                                op=ALU.add,
                            )
                            nc.scalar.activation(sC[:, :, :], cnew[:, :, :], AF.Tanh)
                            if to_stack is None:
                                h16 = sp.tile([128, NKH, BL], BF16, tag=hname, bufs=2)
                                out_ap = h16[:, :, :]
                            else:
                                h16 = None
                                out_ap = to_stack
                            nc.vector.tensor_tensor(
                                out=out_ap, in0=sD[:, :, :], in1=sC[:, :, :],
                                op=ALU.mult,
                            )
                            if t == T - 1:
                                # also save fp32 h for output
                                hf = h0fin if to_stack is None else h1fin
                                nc.vector.tensor_tensor(
                                    out=hf[:, :, :], in0=sD[:, :, :], in1=sC[:, :, :],
                                    op=ALU.mult,
                                )
                            return h16, cnew

                        h0n16, c0new = lstm_elem(gps, c0prev, "h0n", "c0t")

                        # ---- LSTM layer 1 gates ----
                        g1 = pps.tile([128, NMG, BL], F32, tag="gps", bufs=2)
                        for m in range(NMG):
                            for k in range(NKH):
                                nc.tensor.matmul(
                                    out=g1[:, m, :],
                                    lhsT=wih1_sb[:, k, m * 128 : (m + 1) * 128],
                                    rhs=h0n16[:, k, :],
                                    start=(k == 0),
                                    stop=False,
                                )
                            for k in range(NKH):
                                nc.tensor.matmul(
                                    out=g1[:, m, :],
                                    lhsT=whh1_sb[:, k, m * 128 : (m + 1) * 128],
                                    rhs=x_prev[:, k, :],
                                    start=False,
                                    stop=(k == NKH - 1),
                                )
                        nc.vector.tensor_tensor(
                            out=g1[:, :, :],
                            in0=g1[:, :, :],
                            in1=bias1_sb[:, :, None].to_broadcast([128, NMG, BL]),
                            op=ALU.add,
                        )
                        _, c1new = lstm_elem(
                            g1, c1prev, "h1n", "c1t", to_stack=stackT[:, 0:NKH, t, :]
                        )

                        # ---- attention: hs = Wh @ h1n ----
                        hsps = pps.tile([128, NKH, BL], F32, tag="hsps")
                        for m in range(NKH):
                            for k in range(NKH):
                                nc.tensor.matmul(
                                    out=hsps[:, m, :],
                                    lhsT=wh_sb[:, k, m * 128 : (m + 1) * 128],
                                    rhs=stackT[:, k, t, :],
                                    start=(k == 0),
                                    stop=(k == NKH - 1),
                                )
                        hsf = sp.tile([128, NKH, BL], F32, tag="hsf", bufs=2)
                        nc.vector.tensor_copy(out=hsf[:, :, :], in_=hsps[:, :, :])

                        # tanh(hs + ann) in bf16
                        th16 = sp.tile([128, NKH, S, BL], BF16, tag="th", bufs=2)
                        for kt in range(NKH):
                            nc.vector.tensor_tensor(
                                out=th16[:, kt, :, :],
                                in0=annT[:, kt, :, :],
                                in1=hsf[:, kt, None, :].to_broadcast([128, S, BL]),
                                op=ALU.add,
                            )
                        nc.scalar.activation(
                            th16[:, :, :, :], th16[:, :, :, :], AF.Tanh
                        )

                        # e[s,b] = Wo . tanh  -> psum [1, S*BL]
                        eps = pps.tile([1, S, BL], F32, tag="eps")
                        for kt in range(NKH):
                            nc.tensor.matmul(
                                out=eps[:, :, :].rearrange("p s b -> p (s b)"),
                                lhsT=wo_sb[:, kt, :],
                                rhs=th16[:, kt, :, :].rearrange("p s b -> p (s b)"),
                                start=(kt == 0),
                                stop=(kt == NKH - 1),
                            )

                        # softmax over s (free dim), per b
                        emax = sp.tile([1, BL], F32, tag="emax", bufs=2)
                        nc.vector.tensor_reduce(
                            out=emax[:, :],
                            in_=eps[:, :, :].rearrange("p s b -> p b s"),
                            axis=mybir.AxisListType.X,
                            op=ALU.max,
                        )
                        negmax = sp.tile([1, BL], F32, tag="negmax", bufs=2)
                        nc.vector.tensor_scalar_mul(
                            out=negmax[:, :], in0=emax[:, :], scalar1=-1.0
                        )
                        exs = sp.tile([1, S, BL], F32, tag="exs", bufs=2)
                        esum = sp.tile([1, BL], F32, tag="esum", bufs=2)
                        for b in range(BL):
                            nc.scalar.activation(
                                exs[:, :, b],
                                eps[:, :, b],
                                AF.Exp,
                                bias=negmax[:, b : b + 1],
                                accum_out=esum[:, b : b + 1],
                            )
                        rec = sp.tile([1, BL], F32, tag="rec", bufs=2)
                        nc.vector.reciprocal(rec[:, :], esum[:, :])
                        a16 = sp.tile([1, S, BL], BF16, tag="a16", bufs=2)
                        nc.vector.tensor_tensor(
                            out=a16[:, :, :],
                            in0=exs[:, :, :],
                            in1=rec[:, None, :].to_broadcast([1, S, BL]),
                            op=ALU.mult,
                        )
                        # transpose a -> [S, BL] via PE
                        atps = pps.tile([S, BL, 2], BF16, tag="atps")
                        for b in range(BL):
                            nc.tensor.transpose(
                                out=atps[:, b, 0:1],
                                in_=a16[:, :, b],
                                identity=ident16[0:1, 0:1],
                            )
                        aT16 = sp.tile([S, BL], BF16, tag="aT16", bufs=2)
                        nc.vector.tensor_copy(out=aT16[:, :], in_=atps[:, :, 0])

                        # ctx[d, b] = sum_s es[s,b,d] * a[s,b]
                        ctxps = pps.tile([128, NKC, BL], F32, tag="ctxps")
                        for b in range(BL):
                            for dt in range(NKC):
                                nc.tensor.matmul(
                                    out=ctxps[:, dt, b : b + 1],
                                    lhsT=es_sb[:, b, dt, :],
                                    rhs=aT16[:, b : b + 1],
                                    start=True,
                                    stop=True,
                                )
                        nc.vector.tensor_copy(
                            out=stackT[:, NKH:NKP, t, :], in_=ctxps[:, :, :]
                        )

                        h0prev = h0n16
                        c0prev, c1prev = c0new, c1new

                    # save final c states
                    nc.vector.tensor_copy(out=c0fin[:, :, :], in_=c0prev[:, :, :])
                    nc.vector.tensor_copy(out=c1fin[:, :, :], in_=c1prev[:, :, :])

                    # -------- pen.T = Wp @ stacked.T + bp --------
                    for m in range(NME):
                        ps = pps.tile([128, TOK], F32, tag="penps")
                        for k in range(NKP):
                            nc.tensor.matmul(
                                out=ps[:, :],
                                lhsT=wp_sb[:, k, m * 128 : (m + 1) * 128],
                                rhs=stackT[:, k, :, :].rearrange("p t b -> p (t b)"),
                                start=(k == 0),
                                stop=(k == NKP - 1),
                            )
                        nc.vector.tensor_scalar_add(
                            out=penT16[:, m, :],
                            in0=ps[:, :],
                            scalar1=bp_sb[:, m : m + 1],
                        )

            # -------- DMA out h, c --------
            for src, dst in ((h0fin, d_h[0]), (h1fin, d_h[1]),
                             (c0fin, d_c[0]), (c1fin, d_c[1])):
                nc.sync.dma_start(
                    out=dst[:, :, :].rearrange("k p b -> p k b"),
                    in_=src[:, :, :],
                )

            # -------- vocab projection + log_softmax --------
            with tc.tile_pool(name="vocab", bufs=1) as vp, tc.tile_pool(
                name="vocabps", bufs=1, space="PSUM"
            ) as vpp:
                bout_sb = vp.tile([1, V], BF16)
                nc.sync.dma_start(out=bout_sb[:, :], in_=d_bout16[:, :])
                ones16 = vp.tile([1, 128], BF16)
                nc.vector.memset(ones16[:, :], 1.0)
                lbuf = vp.tile([128, V], BF16)
                for mt in range(2):
                    ssum = vp.tile([128, 1], F32, tag="ssum", bufs=2)
                    nc.vector.memset(ssum[:, :], 0.0)
                    for vt, (v0, vn) in enumerate(VCH):
                        et = vp.tile([128, NME, 512], BF16, tag="et", bufs=4)
                        nc.sync.dma_start(
                            out=et[:, :, :vn],
                            in_=d_embT16[:, v0 : v0 + vn].rearrange(
                                "(k p) v -> p k v", p=128
                            ),
                        )
                        ps = vpp.tile([128, 512], F32, tag="vps", bufs=4)
                        for k in range(NME):
                            nc.tensor.matmul(
                                out=ps[:, :vn],
                                lhsT=penT16[:, k, mt * 128 : (mt + 1) * 128],
                                rhs=et[:, k, :vn],
                                start=(k == 0),
                                stop=False,
                            )
                        # += ones^T @ b_out chunk  (adds b_out to every token row)
                        nc.tensor.matmul(
                            out=ps[:, :vn],
                            lhsT=ones16[:, :],
                            rhs=bout_sb[:, v0 : v0 + vn],
                            start=False,
                            stop=True,
                        )
                        nc.vector.tensor_copy(
                            out=lbuf[:, v0 : v0 + vn], in_=ps[:, :vn]
                        )
                        edump = vp.tile([128, 512], BF16, tag="edump", bufs=2)
                        cs = vp.tile([128, 1], F32, tag="cs", bufs=2)
                        nc.scalar.activation(
                            edump[:, :vn], lbuf[:, v0 : v0 + vn], AF.Exp,
                            accum_out=cs[:, :],
                        )
                        nc.vector.tensor_tensor(
                            out=ssum[:, :], in0=ssum[:, :], in1=cs[:, :], op=ALU.add
                        )
                    nlog = vp.tile([128, 1], F32, tag="nlog", bufs=2)
                    nc.scalar.activation(nlog[:, :], ssum[:, :], AF.Ln)
                    nc.vector.tensor_scalar_mul(
                        out=nlog[:, :], in0=nlog[:, :], scalar1=-1.0
                    )
                    for vt, (v0, vn) in enumerate(VCH):
                        stg = vp.tile([128, 512], F32, tag="stg", bufs=4)
                        nc.scalar.activation(
                            stg[:, :vn], lbuf[:, v0 : v0 + vn], AF.Identity,
                            bias=nlog[:, :],
                        )
                        nc.sync.dma_start(
                            out=d_scores[mt * 128 : (mt + 1) * 128, v0 : v0 + vn],
                            in_=stg[:, :vn],
                        )

    nc.finalize()
    return nc


_NC_CACHE = {}


def _get_nc():
    if "nc" not in _NC_CACHE:
        _NC_CACHE["nc"] = build_nc()
    return _NC_CACHE["nc"]


def make_in_maps(trg, encoded_src, emb, W_ih0, W_hh0, b_ih0, b_hh0,
                 W_ih1, W_hh1, b_ih1, b_hh1, Wa, ba, Wh, bh, Wo, bo,
                 Wp, bp, b_out):
    f32 = np.float32
    emb = np.asarray(emb, f32)
    emb16 = emb.astype(bf)
    embT16 = np.ascontiguousarray(emb.T).astype(bf)
    bout16 = np.asarray(b_out, f32).reshape(1, V).astype(bf)
    W_ih0 = np.asarray(W_ih0, f32)
    wembT = np.ascontiguousarray(W_ih0[:, :E].T).astype(bf)
    wctxT = np.ascontiguousarray(W_ih0[:, E:].T).astype(bf)
    whh0T = np.ascontiguousarray(np.asarray(W_hh0, f32).T).astype(bf)
    wih1T = np.ascontiguousarray(np.asarray(W_ih1, f32).T).astype(bf)
    whh1T = np.ascontiguousarray(np.asarray(W_hh1, f32).T).astype(bf)
    whT = np.ascontiguousarray(np.asarray(Wh, f32).T).astype(bf)
    waT = np.ascontiguousarray(np.asarray(Wa, f32).T).astype(bf)
    woT = np.ascontiguousarray(np.asarray(Wo, f32).reshape(1, H).T).astype(bf)
    wpT = np.ascontiguousarray(np.asarray(Wp, f32).T).astype(bf)

    def colmajor(v, nm):
        return np.ascontiguousarray(
            np.asarray(v, f32).reshape(nm, 128).T
        ).astype(f32)

    bias0 = colmajor(np.asarray(b_ih0, f32) + np.asarray(b_hh0, f32), NMG)
    bias1 = colmajor(np.asarray(b_ih1, f32) + np.asarray(b_hh1, f32), NMG)
    biasa = colmajor(np.asarray(ba, f32) + np.asarray(bh, f32), NKH)
    bp_a = colmajor(np.asarray(bp, f32), NME)

    trg = np.asarray(trg)
    encoded_src = np.asarray(encoded_src, f32)

    in_maps = []
    for c in range(NCORES):
        bsl = slice(c * BL, (c + 1) * BL)
        # token index per (t, b): tok = t*BL + b ; idx[p, mt] = tok mt*128+p
        toks = trg[:, bsl].astype(np.int32).reshape(TOK)
        idx = np.ascontiguousarray(toks.reshape(2, 128).T)
        es = np.ascontiguousarray(encoded_src[:, bsl, :])          # [S, BL, DC]
        es16 = es.astype(bf)
        esT16 = np.ascontiguousarray(
            es.transpose(2, 0, 1).reshape(DC, S * BL)
        ).astype(bf)
        in_maps.append({
            "idx": idx, "emb16": emb16, "embT16": embT16, "bout16": bout16,
            "wctxT": wctxT, "wembT": wembT, "whh0T": whh0T, "wih1T": wih1T,
            "whh1T": whh1T, "whT": whT, "waT": waT, "woT": woT, "wpT": wpT,
            "bias0": bias0, "bias1": bias1, "biasa": biasa, "bp": bp_a,
            "es16": es16, "esT16": esT16,
        })
    return in_maps


def assemble(results):
    scores = np.concatenate(
        [r["scores_o"].reshape(T, BL, V) for r in results], axis=1
    ).astype(np.float32)
    def fix_state(a):  # [2, NKH, 128, BL] -> [2, BL, H]
        return np.ascontiguousarray(a.transpose(0, 3, 1, 2).reshape(2, BL, H))

    h = np.concatenate([fix_state(r["h_o"]) for r in results], axis=1).astype(np.float32)
    c = np.concatenate([fix_state(r["c_o"]) for r in results], axis=1).astype(np.float32)
    return scores, h, c


def kernel(**inputs):
    nc = _get_nc()
    in_maps = make_in_maps(**inputs)
    res = run_bass_kernel_spmd(nc, in_maps, core_ids=list(range(NCORES)))
    return assemble(res.results)


# revision 31
# speedup vs baseline: 1.0133x; 1.0133x over previous
"""Trainium2 Bass kernel for BasicBahdanauAttnDecoder.

Strategy (8 NeuronCores, no collectives):
  - Data-parallel over batch B=32 -> 4 batch elements per core.
  - Each core runs: embedding gather (indirect DMA), annot/P0 precompute,
    the sequential T=64 LSTM+attention scan, penultimate projection,
    vocab projection (tied emb weights) + log_softmax, all locally.
  - Host assembles full outputs by concatenating per-core batch slices.

Numerics: bf16 weights/activations for matmuls (PSUM accumulates fp32),
fp32 cell states and softmax statistics. Vocab logsumexp uses no max-shift
(logits are O(1), exp is safe in fp32).

Layouts ("T" suffix = transposed, contraction dim on partitions):
  - scan activations: [dim -> 128-partition tiles, batch(4) free]
  - stackT16 [128, 12, 64, 4]: per-step [h1n; ctx] bf16 (feeds next step + pen)
  - vocab: penT16 [128E, 4kt, 256tok] stationary, embT bf16 moving.
"""

import os
import numpy as np
import ml_dtypes

import concourse.bass as bass
import concourse.bacc as bacc
import concourse.mybir as mybir
import concourse.tile as tile
from concourse.bass_utils import run_bass_kernel_spmd
from concourse.masks import make_identity

BF16 = mybir.dt.bfloat16
F32 = mybir.dt.float32
I32 = mybir.dt.int32
AF = mybir.ActivationFunctionType
ALU = mybir.AluOpType

T, B, S = 64, 32, 64
V, E, H = 32000, 512, 512
NCORES = 8
BL = B // NCORES          # 4 batch per core
TOK = T * BL              # 256 tokens per core
G = 4 * H                 # 2048 gates
DC = 2 * H                # 1024 ctx dim
NMG = G // 128            # 16
NKC = DC // 128           # 8
NKH = H // 128            # 4
NME = E // 128            # 4
NKP = (3 * H) // 128      # 12
# vocab chunks: 62 x 512 + 1 x 256
VCH = [(i * 512, 512) for i in range(62)] + [(31744, 256)]

bf = ml_dtypes.bfloat16


def build_nc():
    # Bacc (not plain Bass): its finalize() splits multi-sem waits into
    # EventSemaphore instructions, which walrus codegen requires.
    nc = bacc.Bacc("TRN2", target_bir_lowering=False)

    # ---------------- I/O declarations ----------------
    d_idx = nc.declare_dram_parameter("idx", [128, 2], I32, isOutput=False)
    d_emb16 = nc.declare_dram_parameter("emb16", [V, E], BF16, isOutput=False)
    d_embT16 = nc.declare_dram_parameter("embT16", [E, V], BF16, isOutput=False)
    # b_out packed into rows at partitions {0,32,64} (22*512 cols each) so the
    # SBUF tile costs 22KB/partition instead of 62.5KB ([1,V] reserves columns
    # across all partitions); matmul requires base_partition in {0,32,64}
    d_bout16 = nc.declare_dram_parameter("bout16", [65, 11264], BF16, isOutput=False)
    d_wctxT = nc.declare_dram_parameter("wctxT", [DC, G], BF16, isOutput=False)
    d_wembT = nc.declare_dram_parameter("wembT", [E, G], BF16, isOutput=False)
    d_whh0T = nc.declare_dram_parameter("whh0T", [H, G], BF16, isOutput=False)
    d_wih1T = nc.declare_dram_parameter("wih1T", [H, G], BF16, isOutput=False)
    d_whh1T = nc.declare_dram_parameter("whh1T", [H, G], BF16, isOutput=False)
    d_whT = nc.declare_dram_parameter("whT", [H, H], BF16, isOutput=False)
    d_waT = nc.declare_dram_parameter("waT", [DC, H], BF16, isOutput=False)
    d_woT = nc.declare_dram_parameter("woT", [H, 1], BF16, isOutput=False)
    d_wpT = nc.declare_dram_parameter("wpT", [3 * H, E], BF16, isOutput=False)
    d_bias0 = nc.declare_dram_parameter("bias0", [128, NMG], F32, isOutput=False)
    d_bias1 = nc.declare_dram_parameter("bias1", [128, NMG], F32, isOutput=False)
    d_biasa = nc.declare_dram_parameter("biasa", [128, NKH], F32, isOutput=False)
    d_bp = nc.declare_dram_parameter("bp", [128, NME], F32, isOutput=False)
    d_es16 = nc.declare_dram_parameter("es16", [S, BL, DC], BF16, isOutput=False)
    d_esT16 = nc.declare_dram_parameter("esT16", [DC, S * BL], BF16, isOutput=False)

    d_scores = nc.declare_dram_parameter("scores_o", [TOK, V], F32, isOutput=True)
    # stored [layer, ktile, partition, b]; host reorders to [2, BL, H]
    d_h = nc.declare_dram_parameter("h_o", [2, NKH, 128, BL], F32, isOutput=True)
    d_c = nc.declare_dram_parameter("c_o", [2, NKH, 128, BL], F32, isOutput=True)

    with tile.TileContext(nc) as tc:
        with tc.tile_pool(name="persist", bufs=1) as pp:
            ident16 = pp.tile([128, 128], BF16)
            make_identity(nc, ident16[:, :])
            penT16 = pp.tile([128, NME, TOK], BF16)
            # final states (bf16 h, fp32 c) saved here at t=T-1
            h0fin = pp.tile([128, NKH, BL], F32)
            h1fin = pp.tile([128, NKH, BL], F32)
            c0fin = pp.tile([128, NKH, BL], F32)
            c1fin = pp.tile([128, NKH, BL], F32)

            with tc.tile_pool(name="scanw", bufs=1) as wp:
                # -------- load weights / constants into SBUF --------
                def load_w(dram, nk, ncol, tag):
                    t_ = wp.tile([128, nk, ncol], BF16, tag=tag)
                    nc.sync.dma_start(
                        out=t_[:, :, :],
                        in_=dram[:, :].rearrange("(k p) g -> p k g", p=128),
                    )
                    return t_

                wctx_sb = load_w(d_wctxT, NKC, G, "wctx")
                wemb_sb = load_w(d_wembT, NME, G, "wemb")
                whh0_sb = load_w(d_whh0T, NKH, G, "whh0")
                wih1_sb = load_w(d_wih1T, NKH, G, "wih1")
                whh1_sb = load_w(d_whh1T, NKH, G, "whh1")
                wh_sb = load_w(d_whT, NKH, H, "wh")
                wa_sb = load_w(d_waT, NKC, H, "wa")
                wo_sb = load_w(d_woT, NKH, 1, "wo")
                wp_sb = load_w(d_wpT, NKP, E, "wp")

                # stage small pointer-operand tiles through DVE so downstream
                # TensorScalarPtr/Activation ops carry fewer sync waits
                def load_small(dram, ncol, tag):
                    raw = wp.tile([128, ncol], F32, tag=tag + "_r")
                    nc.sync.dma_start(out=raw[:, :], in_=dram[:, :])
                    st = wp.tile([128, ncol], F32, tag=tag)
                    nc.vector.tensor_copy(out=st[:, :], in_=raw[:, :])
                    return st

                bias0_sb = load_small(d_bias0, NMG, "bias0")
                bias1_sb = load_small(d_bias1, NMG, "bias1")
                biasa_sb = load_small(d_biasa, NKH, "biasa")
                bp_sb = load_small(d_bp, NME, "bp")

                idx_sb = wp.tile([128, 2], I32)
                nc.sync.dma_start(out=idx_sb[:, :], in_=d_idx[:, :])
                es_sb = wp.tile([S, BL, NKC, 128], BF16)
                nc.sync.dma_start(
                    out=es_sb[:, :, :, :],
                    in_=d_es16[:, :, :].rearrange("s b (k d) -> s b k d", d=128),
                )
                esT_sb = wp.tile([128, NKC, S * BL], BF16)
                nc.sync.dma_start(
                    out=esT_sb[:, :, :],
                    in_=d_esT16[:, :].rearrange("(k p) n -> p k n", p=128),
                )

                # -------- precompute: gather + transpose trg_emb --------
                trg16 = wp.tile([128, 2, E], BF16)
                for mt in range(2):
                    nc.gpsimd.indirect_dma_start(
                        out=trg16[:, mt, :],
                        out_offset=None,
                        in_=d_emb16[:, :],
                        in_offset=bass.IndirectOffsetOnAxis(
                            ap=idx_sb[:, mt : mt + 1], axis=0
                        ),
                    )
                trgT16 = wp.tile([128, NME, TOK], BF16)
                with tc.tile_pool(name="preps", bufs=2, space="PSUM") as prep:
                    for mt in range(2):
                        for ke in range(NME):
                            tp = prep.tile([128, 128], BF16, tag="tp")
                            nc.tensor.transpose(
                                out=tp[:, :],
                                in_=trg16[:, mt, ke * 128 : (ke + 1) * 128],
                                identity=ident16[:, :],
                            )
                            nc.vector.tensor_copy(
                                out=trgT16[:, ke, mt * 128 : (mt + 1) * 128],
                                in_=tp[:, :],
                            )

                    # -------- P0[t] = trg_emb @ W_emb.T + b0  (fp32) --------
                    p0 = wp.tile([128, NMG, T, BL], F32)
                    for m in range(NMG):
                        ps = prep.tile([128, TOK], F32, tag="p0ps")
                        for k in range(NME):
                            nc.tensor.matmul(
                                out=ps[:, :],
                                lhsT=wemb_sb[:, k, m * 128 : (m + 1) * 128],
                                rhs=trgT16[:, k, :],
                                start=(k == 0),
                                stop=(k == NME - 1),
                            )
                        nc.vector.tensor_scalar_add(
                            out=p0[:, m, :, :].rearrange("p t b -> p (t b)"),
                            in0=ps[:, :],
                            scalar1=bias0_sb[:, m : m + 1],
                        )

                    # -------- annT = Wa @ es.T + (ba + bh)  (fp32) --------
                    annT = wp.tile([128, NKH, S, BL], F32)
                    for m in range(NKH):
                        ps = prep.tile([128, S * BL], F32, tag="aps")
                        for k in range(NKC):
                            nc.tensor.matmul(
                                out=ps[:, :],
                                lhsT=wa_sb[:, k, m * 128 : (m + 1) * 128],
                                rhs=esT_sb[:, k, :],
                                start=(k == 0),
                                stop=(k == NKC - 1),
                            )
                        nc.vector.tensor_scalar_add(
                            out=annT[:, m, :, :].rearrange("p s b -> p (s b)"),
                            in0=ps[:, :],
                            scalar1=biasa_sb[:, m : m + 1],
                        )

                # -------- the scan --------
                stackT = wp.tile([128, NKP, T, BL], BF16)
                zeros16 = wp.tile([128, NKP, BL], BF16)
                nc.vector.memset(zeros16[:, :, :], 0.0)
                h0z = wp.tile([128, NKH, BL], BF16)
                nc.vector.memset(h0z[:, :, :], 0.0)
                czero = wp.tile([128, NKH, BL], F32)
                nc.vector.memset(czero[:, :, :], 0.0)

                h0prev = h0z
                c0prev, c1prev = czero, czero

                with tc.tile_pool(name="scansb", bufs=1) as sp, tc.tile_pool(
                    name="scanps", bufs=1, space="PSUM"
                ) as pps:
                    for t in range(T):
                        if t == 0:
                            x_prev = zeros16  # [128, 12, BL]: h1|ctx all zero
                        else:
                            x_prev = stackT[:, :, t - 1, :]

                        # ---- LSTM layer 0 gates ----
                        gps = pps.tile([128, NMG, BL], F32, tag="gps", bufs=2)
                        for m in range(NMG):
                            for k in range(NKC):
                                nc.tensor.matmul(
                                    out=gps[:, m, :],
                                    lhsT=wctx_sb[:, k, m * 128 : (m + 1) * 128],
                                    rhs=x_prev[:, NKH + k, :],
                                    start=(k == 0),
                                    stop=False,
                                )
                            for k in range(NKH):
                                nc.tensor.matmul(
                                    out=gps[:, m, :],
                                    lhsT=whh0_sb[:, k, m * 128 : (m + 1) * 128],
                                    rhs=h0prev[:, k, :],
                                    start=False,
                                    stop=(k == NKH - 1),
                                )
                        nc.vector.tensor_tensor(
                            out=gps[:, :, :],
                            in0=gps[:, :, :],
                            in1=p0[:, :, t, :],
                            op=ALU.add,
                        )

                        def lstm_elem(gtile, cprev, hname, cname, to_stack=None):
                            # gtile [128, 16, BL] psum (i,f,g,o); returns (h16, cnew)
                            sA = sp.tile([128, NKH, BL], F32, tag="sA", bufs=2)
                            sB = sp.tile([128, NKH, BL], F32, tag="sB", bufs=2)
                            sC = sp.tile([128, NKH, BL], F32, tag="sC", bufs=2)
                            sD = sp.tile([128, NKH, BL], F32, tag="sD", bufs=2)
                            nc.scalar.activation(sA[:, :, :], gtile[:, 0:4, :], AF.Sigmoid)
                            nc.scalar.activation(sB[:, :, :], gtile[:, 4:8, :], AF.Sigmoid)
                            nc.scalar.activation(sC[:, :, :], gtile[:, 8:12, :], AF.Tanh)
                            nc.scalar.activation(sD[:, :, :], gtile[:, 12:16, :], AF.Sigmoid)
                            nc.vector.tensor_tensor(
                                out=sA[:, :, :], in0=sA[:, :, :], in1=sC[:, :, :],
                                op=ALU.mult,
                            )
                            cnew = sp.tile([128, NKH, BL], F32, tag=cname, bufs=2)
                            nc.vector.tensor_tensor(
                                out=cnew[:, :, :], in0=sB[:, :, :], in1=cprev[:, :, :],
                                op=ALU.mult,
                            )
                            nc.vector.tensor_tensor(
                                out=cnew[:, :, :], in0=cnew[:, :, :], in1=sA[:, :, :],
                                op=ALU.add,
                            )
                            nc.scalar.activation(sC[:, :, :], cnew[:, :, :], AF.Tanh)
                            if to_stack is None:
                                h16 = sp.tile([128, NKH, BL], BF16, tag=hname, bufs=2)
                                out_ap = h16[:, :, :]
                            else:
                                h16 = None
                                out_ap = to_stack
                            nc.vector.tensor_tensor(
                                out=out_ap, in0=sD[:, :, :], in1=sC[:, :, :],
                                op=ALU.mult,
                            )
                            if t == T - 1:
                                # also save fp32 h for output
                                hf = h0fin if to_stack is None else h1fin
                                nc.vector.tensor_tensor(
                                    out=hf[:, :, :], in0=sD[:, :, :], in1=sC[:, :, :],
                                    op=ALU.mult,
                                )
                            return h16, cnew

                        h0n16, c0new = lstm_elem(gps, c0prev, "h0n", "c0t")

                        # ---- LSTM layer 1 gates ----
                        g1 = pps.tile([128, NMG, BL], F32, tag="gps", bufs=2)
                        for m in range(NMG):
                            for k in range(NKH):
                                nc.tensor.matmul(
                                    out=g1[:, m, :],
                                    lhsT=wih1_sb[:, k, m * 128 : (m + 1) * 128],
                                    rhs=h0n16[:, k, :],
                                    start=(k == 0),
                                    stop=False,
                                )
                            for k in range(NKH):
                                nc.tensor.matmul(
                                    out=g1[:, m, :],
                                    lhsT=whh1_sb[:, k, m * 128 : (m + 1) * 128],
                                    rhs=x_prev[:, k, :],
                                    start=False,
                                    stop=(k == NKH - 1),
                                )
                        nc.vector.tensor_tensor(
                            out=g1[:, :, :],
                            in0=g1[:, :, :],
                            in1=bias1_sb[:, :, None].to_broadcast([128, NMG, BL]),
                            op=ALU.add,
                        )
                        _, c1new = lstm_elem(
                            g1, c1prev, "h1n", "c1t", to_stack=stackT[:, 0:NKH, t, :]
                        )

                        # ---- attention: hs = Wh @ h1n ----
                        hsps = pps.tile([128, NKH, BL], F32, tag="hsps")
                        for m in range(NKH):
                            for k in range(NKH):
                                nc.tensor.matmul(
                                    out=hsps[:, m, :],
                                    lhsT=wh_sb[:, k, m * 128 : (m + 1) * 128],
                                    rhs=stackT[:, k, t, :],
                                    start=(k == 0),
                                    stop=(k == NKH - 1),
                                )
                        hsf = sp.tile([128, NKH, BL], F32, tag="hsf", bufs=2)
                        nc.vector.tensor_copy(out=hsf[:, :, :], in_=hsps[:, :, :])

                        # tanh(hs + ann) in bf16
                        th16 = sp.tile([128, NKH, S, BL], BF16, tag="th", bufs=2)
                        for kt in range(NKH):
                            nc.vector.tensor_tensor(
                                out=th16[:, kt, :, :],
                                in0=annT[:, kt, :, :],
                                in1=hsf[:, kt, None, :].to_broadcast([128, S, BL]),
                                op=ALU.add,
                            )
                        nc.scalar.activation(
                            th16[:, :, :, :], th16[:, :, :, :], AF.Tanh
                        )

                        # e[s,b] = Wo . tanh  -> psum [1, S*BL]
                        eps = pps.tile([1, S, BL], F32, tag="eps")
                        for kt in range(NKH):
                            nc.tensor.matmul(
                                out=eps[:, :, :].rearrange("p s b -> p (s b)"),
                                lhsT=wo_sb[:, kt, :],
                                rhs=th16[:, kt, :, :].rearrange("p s b -> p (s b)"),
                                start=(kt == 0),
                                stop=(kt == NKH - 1),
                            )

                        # softmax over s (free dim), per b
                        emax = sp.tile([1, BL], F32, tag="emax", bufs=2)
                        nc.vector.tensor_reduce(
                            out=emax[:, :],
                            in_=eps[:, :, :].rearrange("p s b -> p b s"),
                            axis=mybir.AxisListType.X,
                            op=ALU.max,
                        )
                        negmax = sp.tile([1, BL], F32, tag="negmax", bufs=2)
                        nc.vector.tensor_scalar_mul(
                            out=negmax[:, :], in0=emax[:, :], scalar1=-1.0
                        )
                        exs = sp.tile([1, S, BL], F32, tag="exs", bufs=2)
                        esum = sp.tile([1, BL], F32, tag="esum", bufs=2)
                        for b in range(BL):
                            nc.scalar.activation(
                                exs[:, :, b],
                                eps[:, :, b],
                                AF.Exp,
                                bias=negmax[:, b : b + 1],
                                accum_out=esum[:, b : b + 1],
                            )
                        rec = sp.tile([1, BL], F32, tag="rec", bufs=2)
                        nc.vector.reciprocal(rec[:, :], esum[:, :])
                        a16 = sp.tile([1, S, BL], BF16, tag="a16", bufs=2)
                        nc.vector.tensor_tensor(
                            out=a16[:, :, :],
                            in0=exs[:, :, :],
                            in1=rec[:, None, :].to_broadcast([1, S, BL]),
                            op=ALU.mult,
                        )
                        # transpose a -> [S, BL] via PE
                        atps = pps.tile([S, BL, 2], BF16, tag="atps")
                        for b in range(BL):
                            nc.tensor.transpose(
                                out=atps[:, b, 0:1],
                                in_=a16[:, :, b],
                                identity=ident16[0:1, 0:1],
                            )
                        aT16 = sp.tile([S, BL], BF16, tag="aT16", bufs=2)
                        nc.vector.tensor_copy(out=aT16[:, :], in_=atps[:, :, 0])

                        # ctx[d, b] = sum_s es[s,b,d] * a[s,b]
                        ctxps = pps.tile([128, NKC, BL], F32, tag="ctxps")
                        for b in range(BL):
                            for dt in range(NKC):
                                nc.tensor.matmul(
                                    out=ctxps[:, dt, b : b + 1],
                                    lhsT=es_sb[:, b, dt, :],
                                    rhs=aT16[:, b : b + 1],
                                    start=True,
                                    stop=True,
                                )
                        nc.vector.tensor_copy(
                            out=stackT[:, NKH:NKP, t, :], in_=ctxps[:, :, :]
                        )

                        h0prev = h0n16
                        c0prev, c1prev = c0new, c1new

                    # save final c states
                    nc.vector.tensor_copy(out=c0fin[:, :, :], in_=c0prev[:, :, :])
                    nc.vector.tensor_copy(out=c1fin[:, :, :], in_=c1prev[:, :, :])

                    # -------- pen.T = Wp @ stacked.T + bp --------
                    for m in range(NME):
                        ps = pps.tile([128, TOK], F32, tag="penps")
                        for k in range(NKP):
                            nc.tensor.matmul(
                                out=ps[:, :],
                                lhsT=wp_sb[:, k, m * 128 : (m + 1) * 128],
                                rhs=stackT[:, k, :, :].rearrange("p t b -> p (t b)"),
                                start=(k == 0),
                                stop=(k == NKP - 1),
                            )
                        nc.vector.tensor_scalar_add(
                            out=penT16[:, m, :],
                            in0=ps[:, :],
                            scalar1=bp_sb[:, m : m + 1],
                        )

            # -------- DMA out h, c --------
            for src, dst in ((h0fin, d_h[0]), (h1fin, d_h[1]),
                             (c0fin, d_c[0]), (c1fin, d_c[1])):
                nc.sync.dma_start(
                    out=dst[:, :, :].rearrange("k p b -> p k b"),
                    in_=src[:, :, :],
                )

            # -------- vocab projection + log_softmax --------
            with tc.tile_pool(name="vocab", bufs=1) as vp, tc.tile_pool(
                name="vocabps", bufs=1, space="PSUM"
            ) as vpp:
                bout_sb = vp.tile([65, 11264], BF16)
                nc.sync.dma_start(out=bout_sb[:, :], in_=d_bout16[:, :])
                ones16 = vp.tile([65, 128], BF16)
                nc.vector.memset(ones16[:, :], 1.0)
                def p1_chunk(mt, lbuf_t, ssum_t, v0, vn):
                    et = vp.tile([128, NME, 512], BF16, tag="et", bufs=4)
                    nc.sync.dma_start(
                        out=et[:, :, :vn],
                        in_=d_embT16[:, v0 : v0 + vn].rearrange(
                            "(k p) v -> p k v", p=128
                        ),
                    )
                    ps = vpp.tile([128, 512], F32, tag="vps", bufs=4)
                    for k in range(NME):
                        nc.tensor.matmul(
                            out=ps[:, :vn],
                            lhsT=penT16[:, k, mt * 128 : (mt + 1) * 128],
                            rhs=et[:, k, :vn],
                            start=(k == 0),
                            stop=False,
                        )
                    # += ones^T @ b_out chunk (adds b_out to every token row)
                    bp_ = 32 * (v0 // 11264)
                    bc_ = v0 % 11264
                    nc.tensor.matmul(
                        out=ps[:, :vn],
                        lhsT=ones16[bp_ : bp_ + 1, :],
                        rhs=bout_sb[bp_ : bp_ + 1, bc_ : bc_ + vn],
                        start=False,
                        stop=True,
                    )
                    nc.vector.tensor_copy(out=lbuf_t[:, v0 : v0 + vn], in_=ps[:, :vn])
                    edump = vp.tile([128, 512], BF16, tag="edump", bufs=2)
                    cs = vp.tile([128, 1], F32, tag="cs", bufs=2)
                    nc.scalar.activation(
                        edump[:, :vn], lbuf_t[:, v0 : v0 + vn], AF.Exp,
                        accum_out=cs[:, :],
                    )
                    nc.vector.tensor_tensor(
                        out=ssum_t[:, :], in0=ssum_t[:, :], in1=cs[:, :], op=ALU.add
                    )

                def finish_norm(ssum_t):
                    nlog = vp.tile([128, 1], F32, tag="nlog", bufs=2)
                    nc.scalar.activation(nlog[:, :], ssum_t[:, :], AF.Ln)
                    nc.vector.tensor_scalar_mul(
                        out=nlog[:, :], in0=nlog[:, :], scalar1=-1.0
                    )
                    return nlog

                def p2_chunk(mt, lbuf_t, nlog_t, v0, vn):
                    stg = vp.tile([128, 512], F32, tag="stg", bufs=4)
                    nc.scalar.activation(
                        stg[:, :vn], lbuf_t[:, v0 : v0 + vn], AF.Identity,
                        bias=nlog_t[:, :],
                    )
                    nc.sync.dma_start(
                        out=d_scores[mt * 128 : (mt + 1) * 128, v0 : v0 + vn],
                        in_=stg[:, :vn],
                    )

                # phase A: pass1 of token group 0
                lbuf0 = vp.tile([128, V], BF16, tag="lbuf", bufs=2)
                ssum0 = vp.tile([128, 1], F32, tag="ssum", bufs=2)
                nc.vector.memset(ssum0[:, :], 0.0)
                for v0, vn in VCH:
                    p1_chunk(0, lbuf0, ssum0, v0, vn)
                nlog0 = finish_norm(ssum0)
                # phase B: pass1(group 1) interleaved with pass2(group 0) so
                # ACT/PE/DMA streams alternate between them instead of
                # serializing (engines execute in emission order)
                lbuf1 = vp.tile([128, V], BF16, tag="lbuf", bufs=2)
                ssum1 = vp.tile([128, 1], F32, tag="ssum", bufs=2)
                nc.vector.memset(ssum1[:, :], 0.0)
                for v0, vn in VCH:
                    p1_chunk(1, lbuf1, ssum1, v0, vn)
                    p2_chunk(0, lbuf0, nlog0, v0, vn)
                nlog1 = finish_norm(ssum1)
                # phase C: pass2 of token group 1
                for v0, vn in VCH:
                    p2_chunk(1, lbuf1, nlog1, v0, vn)

    nc.finalize()
    return nc


_NC_CACHE = {}


def _get_nc():
    if "nc" not in _NC_CACHE:
        _NC_CACHE["nc"] = build_nc()
    return _NC_CACHE["nc"]


def make_in_maps(trg, encoded_src, emb, W_ih0, W_hh0, b_ih0, b_hh0,
                 W_ih1, W_hh1, b_ih1, b_hh1, Wa, ba, Wh, bh, Wo, bo,
                 Wp, bp, b_out):
    f32 = np.float32
    emb = np.asarray(emb, f32)
    emb16 = emb.astype(bf)
    embT16 = np.ascontiguousarray(emb.T).astype(bf)
    bflat = np.asarray(b_out, f32).astype(bf)
    bout16 = np.zeros((65, 11264), dtype=bf)
    for r in range(3):
        seg = bflat[r * 11264 : min((r + 1) * 11264, V)]
        bout16[32 * r, : seg.shape[0]] = seg
    W_ih0 = np.asarray(W_ih0, f32)
    wembT = np.ascontiguousarray(W_ih0[:, :E].T).astype(bf)
    wctxT = np.ascontiguousarray(W_ih0[:, E:].T).astype(bf)
    whh0T = np.ascontiguousarray(np.asarray(W_hh0, f32).T).astype(bf)
    wih1T = np.ascontiguousarray(np.asarray(W_ih1, f32).T).astype(bf)
    whh1T = np.ascontiguousarray(np.asarray(W_hh1, f32).T).astype(bf)
    whT = np.ascontiguousarray(np.asarray(Wh, f32).T).astype(bf)
    waT = np.ascontiguousarray(np.asarray(Wa, f32).T).astype(bf)
    woT = np.ascontiguousarray(np.asarray(Wo, f32).reshape(1, H).T).astype(bf)
    wpT = np.ascontiguousarray(np.asarray(Wp, f32).T).astype(bf)

    def colmajor(v, nm):
        return np.ascontiguousarray(
            np.asarray(v, f32).reshape(nm, 128).T
        ).astype(f32)

    bias0 = colmajor(np.asarray(b_ih0, f32) + np.asarray(b_hh0, f32), NMG)
    bias1 = colmajor(np.asarray(b_ih1, f32) + np.asarray(b_hh1, f32), NMG)
    biasa = colmajor(np.asarray(ba, f32) + np.asarray(bh, f32), NKH)
    bp_a = colmajor(np.asarray(bp, f32), NME)

    trg = np.asarray(trg)
    encoded_src = np.asarray(encoded_src, f32)

    in_maps = []
    for c in range(NCORES):
        bsl = slice(c * BL, (c + 1) * BL)
        # token index per (t, b): tok = t*BL + b ; idx[p, mt] = tok mt*128+p
        toks = trg[:, bsl].astype(np.int32).reshape(TOK)
        idx = np.ascontiguousarray(toks.reshape(2, 128).T)
        es = np.ascontiguousarray(encoded_src[:, bsl, :])          # [S, BL, DC]
        es16 = es.astype(bf)
        esT16 = np.ascontiguousarray(
            es.transpose(2, 0, 1).reshape(DC, S * BL)
        ).astype(bf)
        in_maps.append({
            "idx": idx, "emb16": emb16, "embT16": embT16, "bout16": bout16,
            "wctxT": wctxT, "wembT": wembT, "whh0T": whh0T, "wih1T": wih1T,
            "whh1T": whh1T, "whT": whT, "waT": waT, "woT": woT, "wpT": wpT,
            "bias0": bias0, "bias1": bias1, "biasa": biasa, "bp": bp_a,
            "es16": es16, "esT16": esT16,
        })
    return in_maps


def assemble(results):
    scores = np.concatenate(
        [r["scores_o"].reshape(T, BL, V) for r in results], axis=1
    ).astype(np.float32)
    def fix_state(a):  # [2, NKH, 128, BL] -> [2, BL, H]
        return np.ascontiguousarray(a.transpose(0, 3, 1, 2).reshape(2, BL, H))

    h = np.concatenate([fix_state(r["h_o"]) for r in results], axis=1).astype(np.float32)
    c = np.concatenate([fix_state(r["c_o"]) for r in results], axis=1).astype(np.float32)
    return scores, h, c


def kernel(**inputs):
    nc = _get_nc()
    in_maps = make_in_maps(**inputs)
    res = run_bass_kernel_spmd(nc, in_maps, core_ids=list(range(NCORES)))
    return assemble(res.results)


# revision 32
# speedup vs baseline: 1.0552x; 1.0414x over previous
"""Trainium2 Bass kernel for BasicBahdanauAttnDecoder.

Strategy (8 NeuronCores, no collectives):
  - Data-parallel over batch B=32 -> 4 batch elements per core.
  - Each core runs: embedding gather (indirect DMA), annot/P0 precompute,
    the sequential T=64 LSTM+attention scan, penultimate projection,
    vocab projection (tied emb weights) + log_softmax, all locally.
  - Host assembles full outputs by concatenating per-core batch slices.

Numerics: bf16 weights/activations for matmuls (PSUM accumulates fp32),
fp32 cell states and softmax statistics. Vocab logsumexp uses no max-shift
(logits are O(1), exp is safe in fp32).

Layouts ("T" suffix = transposed, contraction dim on partitions):
  - scan activations: [dim -> 128-partition tiles, batch(4) free]
  - stackT16 [128, 12, 64, 4]: per-step [h1n; ctx] bf16 (feeds next step + pen)
  - vocab: penT16 [128E, 4kt, 256tok] stationary, embT bf16 moving.
"""

import os
import numpy as np
import ml_dtypes

import concourse.bass as bass
import concourse.bacc as bacc
import concourse.mybir as mybir
import concourse.tile as tile
from concourse.bass_utils import run_bass_kernel_spmd
from concourse.masks import make_identity

BF16 = mybir.dt.bfloat16
F32 = mybir.dt.float32
I32 = mybir.dt.int32
AF = mybir.ActivationFunctionType
ALU = mybir.AluOpType

T, B, S = 64, 32, 64
V, E, H = 32000, 512, 512
NCORES = 8
BL = B // NCORES          # 4 batch per core
TOK = T * BL              # 256 tokens per core
G = 4 * H                 # 2048 gates
DC = 2 * H                # 1024 ctx dim
NMG = G // 128            # 16
NKC = DC // 128           # 8
NKH = H // 128            # 4
NME = E // 128            # 4
NKP = (3 * H) // 128      # 12
# vocab chunks: 62 x 512 + 1 x 256
VCH = [(i * 512, 512) for i in range(62)] + [(31744, 256)]

bf = ml_dtypes.bfloat16


def build_nc():
    # Bacc (not plain Bass): its finalize() splits multi-sem waits into
    # EventSemaphore instructions, which walrus codegen requires.
    nc = bacc.Bacc("TRN2", target_bir_lowering=False)

    # ---------------- I/O declarations ----------------
    d_idx = nc.declare_dram_parameter("idx", [128, 2], I32, isOutput=False)
    d_emb16 = nc.declare_dram_parameter("emb16", [V, E], BF16, isOutput=False)
    d_embT16 = nc.declare_dram_parameter("embT16", [E, V], BF16, isOutput=False)
    # b_out packed into rows at partitions {0,32,64} (22*512 cols each) so the
    # SBUF tile costs 22KB/partition instead of 62.5KB ([1,V] reserves columns
    # across all partitions); matmul requires base_partition in {0,32,64}
    d_bout16 = nc.declare_dram_parameter("bout16", [65, 11264], BF16, isOutput=False)
    d_wctxT = nc.declare_dram_parameter("wctxT", [DC, G], BF16, isOutput=False)
    d_wembT = nc.declare_dram_parameter("wembT", [E, G], BF16, isOutput=False)
    d_whh0T = nc.declare_dram_parameter("whh0T", [H, G], BF16, isOutput=False)
    d_wih1T = nc.declare_dram_parameter("wih1T", [H, G], BF16, isOutput=False)
    d_whh1T = nc.declare_dram_parameter("whh1T", [H, G], BF16, isOutput=False)
    d_whT = nc.declare_dram_parameter("whT", [H, H], BF16, isOutput=False)
    d_waT = nc.declare_dram_parameter("waT", [DC, H], BF16, isOutput=False)
    d_woT = nc.declare_dram_parameter("woT", [H, 1], BF16, isOutput=False)
    d_wpT = nc.declare_dram_parameter("wpT", [3 * H, E], BF16, isOutput=False)
    d_bias0 = nc.declare_dram_parameter("bias0", [128, NMG], F32, isOutput=False)
    d_bias1 = nc.declare_dram_parameter("bias1", [128, NMG], F32, isOutput=False)
    d_biasa = nc.declare_dram_parameter("biasa", [128, NKH], F32, isOutput=False)
    d_bp = nc.declare_dram_parameter("bp", [128, NME], F32, isOutput=False)
    d_es16 = nc.declare_dram_parameter("es16", [S, BL, DC], BF16, isOutput=False)
    d_esT16 = nc.declare_dram_parameter("esT16", [DC, S * BL], BF16, isOutput=False)

    d_scores = nc.declare_dram_parameter("scores_o", [TOK, V], F32, isOutput=True)
    # stored [layer, ktile, partition, b]; host reorders to [2, BL, H]
    d_h = nc.declare_dram_parameter("h_o", [2, NKH, 128, BL], F32, isOutput=True)
    d_c = nc.declare_dram_parameter("c_o", [2, NKH, 128, BL], F32, isOutput=True)

    with tile.TileContext(nc) as tc:
        with tc.tile_pool(name="persist", bufs=1) as pp:
            ident16 = pp.tile([128, 128], BF16)
            make_identity(nc, ident16[:, :])
            penT16 = pp.tile([128, NME, TOK], BF16)
            # final states (bf16 h, fp32 c) saved here at t=T-1
            h0fin = pp.tile([128, NKH, BL], F32)
            h1fin = pp.tile([128, NKH, BL], F32)
            c0fin = pp.tile([128, NKH, BL], F32)
            c1fin = pp.tile([128, NKH, BL], F32)

            with tc.tile_pool(name="scanw", bufs=1) as wp:
                # -------- load weights / constants into SBUF --------
                def load_w(dram, nk, ncol, tag):
                    t_ = wp.tile([128, nk, ncol], BF16, tag=tag)
                    nc.sync.dma_start(
                        out=t_[:, :, :],
                        in_=dram[:, :].rearrange("(k p) g -> p k g", p=128),
                    )
                    return t_

                wctx_sb = load_w(d_wctxT, NKC, G, "wctx")
                wemb_sb = load_w(d_wembT, NME, G, "wemb")
                whh0_sb = load_w(d_whh0T, NKH, G, "whh0")
                wih1_sb = load_w(d_wih1T, NKH, G, "wih1")
                whh1_sb = load_w(d_whh1T, NKH, G, "whh1")
                wh_sb = load_w(d_whT, NKH, H, "wh")
                wa_sb = load_w(d_waT, NKC, H, "wa")
                wo_sb = load_w(d_woT, NKH, 1, "wo")
                wp_sb = load_w(d_wpT, NKP, E, "wp")

                # stage small pointer-operand tiles through DVE so downstream
                # TensorScalarPtr/Activation ops carry fewer sync waits
                def load_small(dram, ncol, tag):
                    raw = wp.tile([128, ncol], F32, tag=tag + "_r")
                    nc.sync.dma_start(out=raw[:, :], in_=dram[:, :])
                    st = wp.tile([128, ncol], F32, tag=tag)
                    nc.vector.tensor_copy(out=st[:, :], in_=raw[:, :])
                    return st

                bias0_sb = load_small(d_bias0, NMG, "bias0")
                bias1_sb = load_small(d_bias1, NMG, "bias1")
                biasa_sb = load_small(d_biasa, NKH, "biasa")
                bp_sb = load_small(d_bp, NME, "bp")

                idx_sb = wp.tile([128, 2], I32)
                nc.sync.dma_start(out=idx_sb[:, :], in_=d_idx[:, :])
                es_sb = wp.tile([S, BL, NKC, 128], BF16)
                nc.sync.dma_start(
                    out=es_sb[:, :, :, :],
                    in_=d_es16[:, :, :].rearrange("s b (k d) -> s b k d", d=128),
                )
                esT_sb = wp.tile([128, NKC, S * BL], BF16)
                nc.sync.dma_start(
                    out=esT_sb[:, :, :],
                    in_=d_esT16[:, :].rearrange("(k p) n -> p k n", p=128),
                )

                # -------- precompute: gather + transpose trg_emb --------
                trg16 = wp.tile([128, 2, E], BF16)
                for mt in range(2):
                    nc.gpsimd.indirect_dma_start(
                        out=trg16[:, mt, :],
                        out_offset=None,
                        in_=d_emb16[:, :],
                        in_offset=bass.IndirectOffsetOnAxis(
                            ap=idx_sb[:, mt : mt + 1], axis=0
                        ),
                    )
                trgT16 = wp.tile([128, NME, TOK], BF16)
                with tc.tile_pool(name="preps", bufs=2, space="PSUM") as prep:
                    for mt in range(2):
                        for ke in range(NME):
                            tp = prep.tile([128, 128], BF16, tag="tp")
                            nc.tensor.transpose(
                                out=tp[:, :],
                                in_=trg16[:, mt, ke * 128 : (ke + 1) * 128],
                                identity=ident16[:, :],
                            )
                            nc.vector.tensor_copy(
                                out=trgT16[:, ke, mt * 128 : (mt + 1) * 128],
                                in_=tp[:, :],
                            )

                    # -------- P0[t] = trg_emb @ W_emb.T + b0  (fp32) --------
                    p0 = wp.tile([128, NMG, T, BL], F32)
                    for m in range(NMG):
                        ps = prep.tile([128, TOK], F32, tag="p0ps")
                        for k in range(NME):
                            nc.tensor.matmul(
                                out=ps[:, :],
                                lhsT=wemb_sb[:, k, m * 128 : (m + 1) * 128],
                                rhs=trgT16[:, k, :],
                                start=(k == 0),
                                stop=(k == NME - 1),
                            )
                        nc.vector.tensor_scalar_add(
                            out=p0[:, m, :, :].rearrange("p t b -> p (t b)"),
                            in0=ps[:, :],
                            scalar1=bias0_sb[:, m : m + 1],
                        )

                    # -------- annT = Wa @ es.T + (ba + bh)  (fp32) --------
                    annT = wp.tile([128, NKH, S, BL], F32)
                    for m in range(NKH):
                        ps = prep.tile([128, S * BL], F32, tag="aps")
                        for k in range(NKC):
                            nc.tensor.matmul(
                                out=ps[:, :],
                                lhsT=wa_sb[:, k, m * 128 : (m + 1) * 128],
                                rhs=esT_sb[:, k, :],
                                start=(k == 0),
                                stop=(k == NKC - 1),
                            )
                        nc.vector.tensor_scalar_add(
                            out=annT[:, m, :, :].rearrange("p s b -> p (s b)"),
                            in0=ps[:, :],
                            scalar1=biasa_sb[:, m : m + 1],
                        )

                # -------- the scan --------
                stackT = wp.tile([128, NKP, T, BL], BF16)
                zeros16 = wp.tile([128, NKP, BL], BF16)
                nc.vector.memset(zeros16[:, :, :], 0.0)
                h0z = wp.tile([128, NKH, BL], BF16)
                nc.vector.memset(h0z[:, :, :], 0.0)
                czero = wp.tile([128, NKH, BL], F32)
                nc.vector.memset(czero[:, :, :], 0.0)

                h0prev = h0z
                c0prev, c1prev = czero, czero

                with tc.tile_pool(name="scansb", bufs=1) as sp, tc.tile_pool(
                    name="scanps", bufs=1, space="PSUM"
                ) as pps:
                    for t in range(T):
                        if t == 0:
                            x_prev = zeros16  # [128, 12, BL]: h1|ctx all zero
                        else:
                            x_prev = stackT[:, :, t - 1, :]

                        # ---- LSTM layer 0 gates ----
                        gps = pps.tile([128, NMG, BL], F32, tag="gps", bufs=2)
                        for m in range(NMG):
                            for k in range(NKC):
                                nc.tensor.matmul(
                                    out=gps[:, m, :],
                                    lhsT=wctx_sb[:, k, m * 128 : (m + 1) * 128],
                                    rhs=x_prev[:, NKH + k, :],
                                    start=(k == 0),
                                    stop=False,
                                )
                            for k in range(NKH):
                                nc.tensor.matmul(
                                    out=gps[:, m, :],
                                    lhsT=whh0_sb[:, k, m * 128 : (m + 1) * 128],
                                    rhs=h0prev[:, k, :],
                                    start=False,
                                    stop=(k == NKH - 1),
                                )
                        nc.vector.tensor_tensor(
                            out=gps[:, :, :],
                            in0=gps[:, :, :],
                            in1=p0[:, :, t, :],
                            op=ALU.add,
                        )

                        def lstm_elem(gtile, cprev, hname, cname, to_stack=None):
                            # gtile [128, 16, BL] psum (i,f,g,o); returns (h16, cnew)
                            sA = sp.tile([128, NKH, BL], F32, tag="sA", bufs=2)
                            sB = sp.tile([128, NKH, BL], F32, tag="sB", bufs=2)
                            sC = sp.tile([128, NKH, BL], F32, tag="sC", bufs=2)
                            sD = sp.tile([128, NKH, BL], F32, tag="sD", bufs=2)
                            nc.scalar.activation(sA[:, :, :], gtile[:, 0:4, :], AF.Sigmoid)
                            nc.scalar.activation(sB[:, :, :], gtile[:, 4:8, :], AF.Sigmoid)
                            nc.scalar.activation(sC[:, :, :], gtile[:, 8:12, :], AF.Tanh)
                            nc.scalar.activation(sD[:, :, :], gtile[:, 12:16, :], AF.Sigmoid)
                            nc.vector.tensor_tensor(
                                out=sA[:, :, :], in0=sA[:, :, :], in1=sC[:, :, :],
                                op=ALU.mult,
                            )
                            cnew = sp.tile([128, NKH, BL], F32, tag=cname, bufs=2)
                            nc.vector.tensor_tensor(
                                out=cnew[:, :, :], in0=sB[:, :, :], in1=cprev[:, :, :],
                                op=ALU.mult,
                            )
                            nc.vector.tensor_tensor(
                                out=cnew[:, :, :], in0=cnew[:, :, :], in1=sA[:, :, :],
                                op=ALU.add,
                            )
                            nc.scalar.activation(sC[:, :, :], cnew[:, :, :], AF.Tanh)
                            if to_stack is None:
                                h16 = sp.tile([128, NKH, BL], BF16, tag=hname, bufs=2)
                                out_ap = h16[:, :, :]
                            else:
                                h16 = None
                                out_ap = to_stack
                            nc.vector.tensor_tensor(
                                out=out_ap, in0=sD[:, :, :], in1=sC[:, :, :],
                                op=ALU.mult,
                            )
                            if t == T - 1:
                                # also save fp32 h for output
                                hf = h0fin if to_stack is None else h1fin
                                nc.vector.tensor_tensor(
                                    out=hf[:, :, :], in0=sD[:, :, :], in1=sC[:, :, :],
                                    op=ALU.mult,
                                )
                            return h16, cnew

                        h0n16, c0new = lstm_elem(gps, c0prev, "h0n", "c0t")

                        # ---- LSTM layer 1 gates ----
                        g1 = pps.tile([128, NMG, BL], F32, tag="gps", bufs=2)
                        for m in range(NMG):
                            for k in range(NKH):
                                nc.tensor.matmul(
                                    out=g1[:, m, :],
                                    lhsT=wih1_sb[:, k, m * 128 : (m + 1) * 128],
                                    rhs=h0n16[:, k, :],
                                    start=(k == 0),
                                    stop=False,
                                )
                            for k in range(NKH):
                                nc.tensor.matmul(
                                    out=g1[:, m, :],
                                    lhsT=whh1_sb[:, k, m * 128 : (m + 1) * 128],
                                    rhs=x_prev[:, k, :],
                                    start=False,
                                    stop=(k == NKH - 1),
                                )
                        nc.vector.tensor_tensor(
                            out=g1[:, :, :],
                            in0=g1[:, :, :],
                            in1=bias1_sb[:, :, None].to_broadcast([128, NMG, BL]),
                            op=ALU.add,
                        )
                        _, c1new = lstm_elem(
                            g1, c1prev, "h1n", "c1t", to_stack=stackT[:, 0:NKH, t, :]
                        )

                        # ---- attention: hs = Wh @ h1n ----
                        hsps = pps.tile([128, NKH, BL], F32, tag="hsps")
                        for m in range(NKH):
                            for k in range(NKH):
                                nc.tensor.matmul(
                                    out=hsps[:, m, :],
                                    lhsT=wh_sb[:, k, m * 128 : (m + 1) * 128],
                                    rhs=stackT[:, k, t, :],
                                    start=(k == 0),
                                    stop=(k == NKH - 1),
                                )
                        hsf = sp.tile([128, NKH, BL], F32, tag="hsf", bufs=2)
                        nc.vector.tensor_copy(out=hsf[:, :, :], in_=hsps[:, :, :])

                        # tanh(hs + ann) in bf16
                        th16 = sp.tile([128, NKH, S, BL], BF16, tag="th", bufs=2)
                        for kt in range(NKH):
                            nc.vector.tensor_tensor(
                                out=th16[:, kt, :, :],
                                in0=annT[:, kt, :, :],
                                in1=hsf[:, kt, None, :].to_broadcast([128, S, BL]),
                                op=ALU.add,
                            )
                        nc.scalar.activation(
                            th16[:, :, :, :], th16[:, :, :, :], AF.Tanh
                        )

                        # e[s,b] = Wo . tanh  -> psum [1, S*BL]
                        eps = pps.tile([1, S, BL], F32, tag="eps")
                        for kt in range(NKH):
                            nc.tensor.matmul(
                                out=eps[:, :, :].rearrange("p s b -> p (s b)"),
                                lhsT=wo_sb[:, kt, :],
                                rhs=th16[:, kt, :, :].rearrange("p s b -> p (s b)"),
                                start=(kt == 0),
                                stop=(kt == NKH - 1),
                            )

                        # softmax over s (free dim), per b
                        emax = sp.tile([1, BL], F32, tag="emax", bufs=2)
                        nc.vector.tensor_reduce(
                            out=emax[:, :],
                            in_=eps[:, :, :].rearrange("p s b -> p b s"),
                            axis=mybir.AxisListType.X,
                            op=ALU.max,
                        )
                        negmax = sp.tile([1, BL], F32, tag="negmax", bufs=2)
                        nc.vector.tensor_scalar_mul(
                            out=negmax[:, :], in0=emax[:, :], scalar1=-1.0
                        )
                        exs = sp.tile([1, S, BL], F32, tag="exs", bufs=2)
                        esum = sp.tile([1, BL], F32, tag="esum", bufs=2)
                        for b in range(BL):
                            nc.scalar.activation(
                                exs[:, :, b],
                                eps[:, :, b],
                                AF.Exp,
                                bias=negmax[:, b : b + 1],
                                accum_out=esum[:, b : b + 1],
                            )
                        rec = sp.tile([1, BL], F32, tag="rec", bufs=2)
                        nc.vector.reciprocal(rec[:, :], esum[:, :])
                        a16 = sp.tile([1, S, BL], BF16, tag="a16", bufs=2)
                        nc.vector.tensor_tensor(
                            out=a16[:, :, :],
                            in0=exs[:, :, :],
                            in1=rec[:, None, :].to_broadcast([1, S, BL]),
                            op=ALU.mult,
                        )
                        # transpose a -> [S, BL] via PE
                        atps = pps.tile([S, BL, 2], BF16, tag="atps")
                        for b in range(BL):
                            nc.tensor.transpose(
                                out=atps[:, b, 0:1],
                                in_=a16[:, :, b],
                                identity=ident16[0:1, 0:1],
                            )
                        aT16 = sp.tile([S, BL], BF16, tag="aT16", bufs=2)
                        nc.vector.tensor_copy(out=aT16[:, :], in_=atps[:, :, 0])

                        # ctx[d, b] = sum_s es[s,b,d] * a[s,b]
                        ctxps = pps.tile([128, NKC, BL], F32, tag="ctxps")
                        for b in range(BL):
                            for dt in range(NKC):
                                nc.tensor.matmul(
                                    out=ctxps[:, dt, b : b + 1],
                                    lhsT=es_sb[:, b, dt, :],
                                    rhs=aT16[:, b : b + 1],
                                    start=True,
                                    stop=True,
                                )
                        nc.vector.tensor_copy(
                            out=stackT[:, NKH:NKP, t, :], in_=ctxps[:, :, :]
                        )

                        h0prev = h0n16
                        c0prev, c1prev = c0new, c1new

                    # save final c states
                    nc.vector.tensor_copy(out=c0fin[:, :, :], in_=c0prev[:, :, :])
                    nc.vector.tensor_copy(out=c1fin[:, :, :], in_=c1prev[:, :, :])

                    # -------- pen.T = Wp @ stacked.T + bp --------
                    for m in range(NME):
                        ps = pps.tile([128, TOK], F32, tag="penps")
                        for k in range(NKP):
                            nc.tensor.matmul(
                                out=ps[:, :],
                                lhsT=wp_sb[:, k, m * 128 : (m + 1) * 128],
                                rhs=stackT[:, k, :, :].rearrange("p t b -> p (t b)"),
                                start=(k == 0),
                                stop=(k == NKP - 1),
                            )
                        nc.vector.tensor_scalar_add(
                            out=penT16[:, m, :],
                            in0=ps[:, :],
                            scalar1=bp_sb[:, m : m + 1],
                        )

            # -------- DMA out h, c --------
            for src, dst in ((h0fin, d_h[0]), (h1fin, d_h[1]),
                             (c0fin, d_c[0]), (c1fin, d_c[1])):
                nc.sync.dma_start(
                    out=dst[:, :, :].rearrange("k p b -> p k b"),
                    in_=src[:, :, :],
                )

            # -------- vocab projection + log_softmax --------
            with tc.tile_pool(name="vocab", bufs=1) as vp, tc.tile_pool(
                name="vocabps", bufs=1, space="PSUM"
            ) as vpp:
                bout_sb = vp.tile([65, 11264], BF16)
                nc.sync.dma_start(out=bout_sb[:, :], in_=d_bout16[:, :])
                ones16 = vp.tile([65, 128], BF16)
                nc.vector.memset(ones16[:, :], 1.0)
                def p1_chunk(mt, lbuf_t, ssum_t, v0, vn):
                    et = vp.tile([128, NME, 512], BF16, tag="et", bufs=4)
                    nc.sync.dma_start(
                        out=et[:, :, :vn],
                        in_=d_embT16[:, v0 : v0 + vn].rearrange(
                            "(k p) v -> p k v", p=128
                        ),
                    )
                    ps = vpp.tile([128, 512], F32, tag="vps", bufs=4)
                    for k in range(NME):
                        nc.tensor.matmul(
                            out=ps[:, :vn],
                            lhsT=penT16[:, k, mt * 128 : (mt + 1) * 128],
                            rhs=et[:, k, :vn],
                            start=(k == 0),
                            stop=False,
                        )
                    # += ones^T @ b_out chunk (adds b_out to every token row)
                    bp_ = 32 * (v0 // 11264)
                    bc_ = v0 % 11264
                    nc.tensor.matmul(
                        out=ps[:, :vn],
                        lhsT=ones16[bp_ : bp_ + 1, :],
                        rhs=bout_sb[bp_ : bp_ + 1, bc_ : bc_ + vn],
                        start=False,
                        stop=True,
                    )
                    nc.vector.tensor_copy(out=lbuf_t[:, v0 : v0 + vn], in_=ps[:, :vn])
                    edump = vp.tile([128, 512], BF16, tag="edump", bufs=2)
                    cs = vp.tile([128, 1], F32, tag="cs", bufs=2)
                    nc.scalar.activation(
                        edump[:, :vn], lbuf_t[:, v0 : v0 + vn], AF.Exp,
                        accum_out=cs[:, :],
                    )
                    nc.vector.tensor_tensor(
                        out=ssum_t[:, :], in0=ssum_t[:, :], in1=cs[:, :], op=ALU.add
                    )

                def finish_norm(ssum_t):
                    nlog = vp.tile([128, 1], F32, tag="nlog", bufs=2)
                    nc.scalar.activation(nlog[:, :], ssum_t[:, :], AF.Ln)
                    nc.vector.tensor_scalar_mul(
                        out=nlog[:, :], in0=nlog[:, :], scalar1=-1.0
                    )
                    return nlog

                def p2_chunk(mt, lbuf_t, nlog_t, v0, vn):
                    stg = vp.tile([128, 512], F32, tag="stg", bufs=6)
                    nc.scalar.activation(
                        stg[:, :vn], lbuf_t[:, v0 : v0 + vn], AF.Identity,
                        bias=nlog_t[:, :],
                    )
                    # gpsimd queues: keep scores writes off the sync-engine
                    # queues that stream the embT loads
                    nc.gpsimd.dma_start(
                        out=d_scores[mt * 128 : (mt + 1) * 128, v0 : v0 + vn],
                        in_=stg[:, :vn],
                    )

                # phase A: pass1 of token group 0
                lbuf0 = vp.tile([128, V], BF16, tag="lbuf", bufs=2)
                ssum0 = vp.tile([128, 1], F32, tag="ssum", bufs=2)
                nc.vector.memset(ssum0[:, :], 0.0)
                for v0, vn in VCH:
                    p1_chunk(0, lbuf0, ssum0, v0, vn)
                nlog0 = finish_norm(ssum0)
                # phase B: pass1(group 1) interleaved with pass2(group 0) so
                # ACT/PE/DMA streams alternate between them instead of
                # serializing (engines execute in emission order)
                lbuf1 = vp.tile([128, V], BF16, tag="lbuf", bufs=2)
                ssum1 = vp.tile([128, 1], F32, tag="ssum", bufs=2)
                nc.vector.memset(ssum1[:, :], 0.0)
                for v0, vn in VCH:
                    p1_chunk(1, lbuf1, ssum1, v0, vn)
                    p2_chunk(0, lbuf0, nlog0, v0, vn)
                nlog1 = finish_norm(ssum1)
                # phase C: pass2 of token group 1
                for v0, vn in VCH:
                    p2_chunk(1, lbuf1, nlog1, v0, vn)

    nc.finalize()
    return nc


_NC_CACHE = {}


def _get_nc():
    if "nc" not in _NC_CACHE:
        _NC_CACHE["nc"] = build_nc()
    return _NC_CACHE["nc"]


def make_in_maps(trg, encoded_src, emb, W_ih0, W_hh0, b_ih0, b_hh0,
                 W_ih1, W_hh1, b_ih1, b_hh1, Wa, ba, Wh, bh, Wo, bo,
                 Wp, bp, b_out):
    f32 = np.float32
    emb = np.asarray(emb, f32)
    emb16 = emb.astype(bf)
    embT16 = np.ascontiguousarray(emb.T).astype(bf)
    bflat = np.asarray(b_out, f32).astype(bf)
    bout16 = np.zeros((65, 11264), dtype=bf)
    for r in range(3):
        seg = bflat[r * 11264 : min((r + 1) * 11264, V)]
        bout16[32 * r, : seg.shape[0]] = seg
    W_ih0 = np.asarray(W_ih0, f32)
    wembT = np.ascontiguousarray(W_ih0[:, :E].T).astype(bf)
    wctxT = np.ascontiguousarray(W_ih0[:, E:].T).astype(bf)
    whh0T = np.ascontiguousarray(np.asarray(W_hh0, f32).T).astype(bf)
    wih1T = np.ascontiguousarray(np.asarray(W_ih1, f32).T).astype(bf)
    whh1T = np.ascontiguousarray(np.asarray(W_hh1, f32).T).astype(bf)
    whT = np.ascontiguousarray(np.asarray(Wh, f32).T).astype(bf)
    waT = np.ascontiguousarray(np.asarray(Wa, f32).T).astype(bf)
    woT = np.ascontiguousarray(np.asarray(Wo, f32).reshape(1, H).T).astype(bf)
    wpT = np.ascontiguousarray(np.asarray(Wp, f32).T).astype(bf)

    def colmajor(v, nm):
        return np.ascontiguousarray(
            np.asarray(v, f32).reshape(nm, 128).T
        ).astype(f32)

    bias0 = colmajor(np.asarray(b_ih0, f32) + np.asarray(b_hh0, f32), NMG)
    bias1 = colmajor(np.asarray(b_ih1, f32) + np.asarray(b_hh1, f32), NMG)
    biasa = colmajor(np.asarray(ba, f32) + np.asarray(bh, f32), NKH)
    bp_a = colmajor(np.asarray(bp, f32), NME)

    trg = np.asarray(trg)
    encoded_src = np.asarray(encoded_src, f32)

    in_maps = []
    for c in range(NCORES):
        bsl = slice(c * BL, (c + 1) * BL)
        # token index per (t, b): tok = t*BL + b ; idx[p, mt] = tok mt*128+p
        toks = trg[:, bsl].astype(np.int32).reshape(TOK)
        idx = np.ascontiguousarray(toks.reshape(2, 128).T)
        es = np.ascontiguousarray(encoded_src[:, bsl, :])          # [S, BL, DC]
        es16 = es.astype(bf)
        esT16 = np.ascontiguousarray(
            es.transpose(2, 0, 1).reshape(DC, S * BL)
        ).astype(bf)
        in_maps.append({
            "idx": idx, "emb16": emb16, "embT16": embT16, "bout16": bout16,
            "wctxT": wctxT, "wembT": wembT, "whh0T": whh0T, "wih1T": wih1T,
            "whh1T": whh1T, "whT": whT, "waT": waT, "woT": woT, "wpT": wpT,
            "bias0": bias0, "bias1": bias1, "biasa": biasa, "bp": bp_a,
            "es16": es16, "esT16": esT16,
        })
    return in_maps


def assemble(results):
    scores = np.concatenate(
        [r["scores_o"].reshape(T, BL, V) for r in results], axis=1
    ).astype(np.float32)
    def fix_state(a):  # [2, NKH, 128, BL] -> [2, BL, H]
        return np.ascontiguousarray(a.transpose(0, 3, 1, 2).reshape(2, BL, H))

    h = np.concatenate([fix_state(r["h_o"]) for r in results], axis=1).astype(np.float32)
    c = np.concatenate([fix_state(r["c_o"]) for r in results], axis=1).astype(np.float32)
    return scores, h, c


def kernel(**inputs):
    nc = _get_nc()
    in_maps = make_in_maps(**inputs)
    res = run_bass_kernel_spmd(nc, in_maps, core_ids=list(range(NCORES)))
    return assemble(res.results)
